# revision 10
# baseline (speedup 1.0000x reference)
"""CMC loss kernel for Trainium2, sharded across 8 NeuronCores.

Sharding: core i owns view d=i for the negative term (full BxB Gram of
zn[:, i, :]), and the 256-sample slice [256*i, 256*(i+1)) for the positive
term (all 28 view pairs).  Host combines per-core partial sums and does the
final (tiny) logits/logsumexp reduction.

Pipeline (all inputs host-packed bf16):
 - zv chunks DMA'd -> GPSIMD squares -> DVE seg-reduce -> ACT ln/exp inv
 - normalize+transpose fused on PE: matmul(lhsT=z_raw_tile, rhs=diag(inv))
   writes znT = diag-scaled transpose straight into PSUM; DVE copies to SBUF
 - gram rows: 8 bf16 matmuls per 128-row block into [P,2048] PSUM (two
   ping-pong banks-halves), ACT exp(scale=1/T) with accum_out row sums
 - pos term: GPSIMD banded pair products (k-diagonals incl. k=0 squares),
   DVE segmented reduce, ACT inv + exp, DVE scale/reduce
"""

import os
import sys

import numpy as np

sys.path.insert(0, "/opt/trn_rl_repo")

import concourse.bass as bass  # noqa: E402
import concourse.mybir as mybir  # noqa: E402
from concourse.bass_utils import run_bass_kernel_spmd  # noqa: E402
from concourse.library_overlay import lower_extended_insts  # noqa: E402
from concourse.tile import TileContext  # noqa: E402


def _split_waits(nc, max_waits=1):
    """Hoist excess semaphore waits onto standalone event-sem instructions.

    Most TRN2 ISA structs only have sync slots for one wait (plus updates);
    walrus fails with "Too many sync wait commands" otherwise.  An engine
    stalls identically whether a wait rides on the instruction or on an
    InstEventSemaphore immediately before it in the same engine's stream,
    so splitting is semantics-preserving.
    """
    n = 0
    for fn in nc.m.functions:
        for bb in fn.blocks:
            out = []
            changed = False
            for inst in bb.instructions:
                si = inst.sync_info
                if si is not None and si.on_wait and len(si.on_wait) > max_waits:
                    waits = list(si.on_wait)
                    for w in waits[:-max_waits]:
                        out.append(
                            mybir.InstEventSemaphore(
                                name=f"WSPLIT-{n}",
                                engine=inst.engine,
                                ins=[],
                                outs=[],
                                sync_info=mybir.SyncInfo(
                                    on_wait=[w], on_update=[]
                                ),
                            )
                        )
                        n += 1
                    inst.sync_info = mybir.SyncInfo(
                        on_wait=waits[-max_waits:], on_update=si.on_update
                    )
                    changed = True
                out.append(inst)
            if changed:
                bb.instructions = out

B, D, F = 2048, 8, 256
NCORES = 8
BS = B // NCORES  # 256 samples per core (pos term)
P = 128
NB = B // P  # 16 row/b tiles
NF = F // P  # 2 feature halves
CC = 512  # matmul free-dim chunk
NPAIR = D * (D - 1) // 2  # 28 unordered view pairs
NJ = BS // P  # 2 sample tiles for pos term
TEMP = 0.5
INV_TEMP = 1.0 / TEMP

# packed input layout (columns of zin, all bf16):
NZV = NB * F  # 4096: z[:, i, :] as [P, NB, F]
NZS = NJ * D * F  # 4096: z[i*BS:(i+1)*BS] as [P, NJ, D, F]
ZIN_W = NZV + NZS + P  # + 128 identity columns
NBAND = D * (D + 1) // 2  # 36 segments: k-diagonal bands incl. k=0
BAND_OFF = [0]
for _k in range(1, D + 1):
    BAND_OFF.append(BAND_OFF[-1] + (D - _k + 1))  # offsets per k band

f32 = mybir.dt.float32
bf16 = mybir.dt.bfloat16
ALU = mybir.AluOpType
ACT = mybir.ActivationFunctionType

_CACHED_NC = None


def _build_nc():
    nc = bass.Bass()

    zin = nc.dram_tensor("zin", [P, ZIN_W], bf16, kind="ExternalInput")
    # columns 0..NB-1: per-view gram exp row sums (incl. diagonal)
    # columns NB..NB+NJ-1: pos-term pair-exp sums (d<e only)
    out = nc.dram_tensor("out", [P, NB + NJ], f32, kind="ExternalOutput")

    with TileContext(nc) as tc:
        with (
            tc.tile_pool(name="singles", bufs=1) as singles,
            tc.tile_pool(name="work", bufs=3) as work,
            tc.tile_pool(name="small", bufs=4) as small,
            tc.tile_pool(name="psA", bufs=1, space="PSUM") as psA,
            tc.tile_pool(name="psB", bufs=1, space="PSUM") as psB,
        ):
            zv_sb = singles.tile([P, NB, F], bf16)  # raw view slice
            zs_sb = singles.tile([P, NJ, D, F], bf16)  # raw sample slice
            id_sb = singles.tile([P, P], bf16)  # identity
            znT = singles.tile([P, NF, B], bf16)  # normalized transpose
            n2 = singles.tile([P, NB], f32)
            inv = singles.tile([P, NB], f32)

            zv_flat = zv_sb[:, :, :].rearrange("p t f -> p (t f)")

            # --- input DMAs, chunked for pipelining ---
            NCH = 4
            CW = NZV // NCH  # 1024 cols per zv chunk
            for c in range(NCH):
                nc.sync.dma_start(
                    out=zv_flat[:, c * CW : (c + 1) * CW],
                    in_=zin[:, c * CW : (c + 1) * CW],
                )
            nc.sync.dma_start(out=id_sb, in_=zin[:, NZV + NZS :])
            zs_flat = zs_sb[:, :, :, :].rearrange("p j d f -> p (j d f)")
            for j in range(NJ):
                nc.sync.dma_start(
                    out=zs_flat[:, j * D * F : (j + 1) * D * F],
                    in_=zin[:, NZV + j * D * F : NZV + (j + 1) * D * F],
                )

            tpa = psA.tile([P, B], f32, tag="ps")
            tpb = psB.tile([P, B], f32, tag="ps")
            tps = [tpa, tpb]

            # --- per-tile fused square+reduce for norms, then inv ---
            TPC = NB // NCH  # 4 tiles per chunk
            for c in range(NCH):
                sq = work.tile([P, CW], bf16, tag="sq")
                nc.vector.tensor_mul(
                    sq,
                    zv_flat[:, c * CW : (c + 1) * CW],
                    zv_flat[:, c * CW : (c + 1) * CW],
                )
                nc.vector.tensor_reduce(
                    out=n2[:, c * TPC : (c + 1) * TPC],
                    in_=sq[:, :].rearrange("p (t f) -> p t f", t=TPC),
                    axis=mybir.AxisListType.X,
                    op=ALU.add,
                )
                # inv = 1/sqrt(n2) = exp(-0.5 ln n2), per chunk
                lnt = small.tile([P, TPC], f32, tag="lnt")
                nc.scalar.activation(
                    lnt, n2[:, c * TPC : (c + 1) * TPC], ACT.Ln
                )
                nc.scalar.activation(
                    inv[:, c * TPC : (c + 1) * TPC], lnt, ACT.Exp, scale=-0.5
                )
                # normalize (DVE) + transpose (PE, rhs=identity)
                # TP tiles: 16 slices of 128 cols; group g of 4 tiles is
                # h-major: offset = g*1024 + h*512 + (t%4)*128
                for ti in range(TPC):
                    t = c * TPC + ti
                    znb = work.tile([P, F], bf16, tag="znb")
                    nc.vector.tensor_scalar_mul(
                        znb, zv_sb[:, t, :], inv[:, t : t + 1]
                    )
                    tp = tps[t // 8]
                    g = (t // 4) % 2
                    for h in range(NF):
                        off = g * 1024 + h * 512 + (t % 4) * 128
                        nc.tensor.matmul(
                            tp[:, off : off + P],
                            znb[:, h * P : (h + 1) * P],
                            id_sb,
                            start=True,
                            stop=True,
                        )
            for g4 in range(4):
                # copy each finished 4-tile group to znT
                tp = tps[g4 // 2]
                g = g4 % 2
                nc.vector.tensor_copy(
                    znT[:, :, (g4 * 512) : (g4 * 512 + 512)],
                    tp[:, (g4 % 2) * 1024 : (g4 % 2) * 1024 + 1024].rearrange(
                        "p (h c) -> p h c", h=NF
                    ),
                )

            # --- gram rows: 8 MMs per 128-row block + exp/accum ---
            rowsums = singles.tile([P, NB], f32)
            for rb in range(NB):
                ps = (psA if rb % 2 == 0 else psB).tile([P, B], f32, tag="ps")
                for h in range(NF):
                    for cc in range(B // CC):
                        nc.tensor.matmul(
                            ps[:, cc * CC : (cc + 1) * CC],
                            znT[:, h, rb * P : (rb + 1) * P],
                            znT[:, h, cc * CC : (cc + 1) * CC],
                            start=(h == 0),
                            stop=(h == NF - 1),
                        )
                ejunk = work.tile([P, B], bf16, tag="ejunk")
                nc.scalar.activation(
                    ejunk, ps, ACT.Exp, scale=INV_TEMP,
                    accum_out=rowsums[:, rb : rb + 1],
                )

            # --- pos term: banded pair products (flat 2D APs), then
            # segmented reduce; emitted after gram so it fills idle slots ---
            rdots = singles.tile([P, NJ, NBAND], f32)
            for j in range(NJ):
                prodj = work.tile([P, NBAND * F], bf16, tag="prodj")
                eng = nc.gpsimd if j == 0 else nc.vector
                for k in range(D):
                    o = BAND_OFF[k] * F
                    w = (D - k) * F
                    eng.tensor_mul(
                        prodj[:, o : o + w],
                        zs_flat[:, j * D * F : j * D * F + w],
                        zs_flat[:, j * D * F + k * F : (j + 1) * D * F],
                    )
                nc.vector.tensor_reduce(
                    out=rdots[:, j, :],
                    in_=prodj[:, :].rearrange("p (s f) -> p s f", s=NBAND),
                    axis=mybir.AxisListType.X,
                    op=ALU.add,
                )
            # inv_s = 1/sqrt(diag band)
            invs = singles.tile([P, NJ, D], f32)
            lnts = small.tile([P, NJ, D], f32, tag="lnts")
            nc.scalar.activation(lnts, rdots[:, :, 0:D], ACT.Ln)
            nc.scalar.activation(invs, lnts, ACT.Exp, scale=-0.5)
            # ip[j, pair(d, d+k)] = invs[j, d] * invs[j, d+k]
            ip = singles.tile([P, NJ, NPAIR], f32)
            for k in range(1, D):
                o = BAND_OFF[k] - D
                w = D - k
                nc.vector.tensor_tensor(
                    out=ip[:, :, o : o + w],
                    in0=invs[:, :, 0:w],
                    in1=invs[:, :, k:D],
                    op=ALU.mult,
                )
            sd = small.tile([P, NJ, NPAIR], f32, tag="sd")
            nc.vector.tensor_tensor(
                out=sd, in0=rdots[:, :, D:], in1=ip, op=ALU.mult
            )
            ed = small.tile([P, NJ, NPAIR], f32, tag="ed")
            nc.scalar.activation(ed, sd, ACT.Exp, scale=INV_TEMP)
            possum = singles.tile([P, NJ], f32)
            nc.vector.tensor_reduce(
                out=possum, in_=ed, axis=mybir.AxisListType.X, op=ALU.add
            )

            # --- outputs ---
            outsb = singles.tile([P, NB + NJ], f32)
            nc.vector.tensor_copy(outsb[:, :NB], rowsums)
            nc.vector.tensor_copy(outsb[:, NB:], possum)
            nc.sync.dma_start(out=out[:, :], in_=outsb)

    if os.environ.get("KERNEL_NO_SPLIT") != "1":  # CoreSim can't run the
        _split_waits(nc)  # post-hoc event-sem instructions; HW needs them
    lower_extended_insts(nc)
    return nc


def _get_nc():
    global _CACHED_NC
    if _CACHED_NC is None:
        _CACHED_NC = _build_nc()
    return _CACHED_NC


def _pack_core_input(zb, ident, i):
    zv = zb[:, i, :].reshape(NB, P, F).transpose(1, 0, 2).reshape(P, NZV)
    zs = (
        zb[i * BS : (i + 1) * BS]
        .reshape(NJ, P, D, F)
        .transpose(1, 0, 2, 3)
        .reshape(P, NZS)
    )
    return np.ascontiguousarray(np.concatenate([zv, zs, ident], axis=1))


def _run(z, trace=False):
    import ml_dtypes

    z = np.asarray(z, dtype=np.float32)
    assert z.shape == (B, D, F), z.shape
    zb = z.astype(ml_dtypes.bfloat16)
    ident = np.eye(P, dtype=ml_dtypes.bfloat16)
    in_maps = [{"zin": _pack_core_input(zb, ident, i)} for i in range(NCORES)]
    nc = _get_nc()
    res = run_bass_kernel_spmd(
        nc, in_maps, core_ids=list(range(NCORES)), trace=trace
    )
    return res


def _finish(results):
    neg_raw = np.zeros(B, np.float64)
    pos_half = np.zeros(B, np.float64)
    for i, r in enumerate(results):
        o = np.asarray(r["out"], np.float64)  # [P, NB + NJ]
        rowsums = o[:, :NB]  # [P, NB] ; sample = t*128 + p
        possums = o[:, NB:]  # [P, NJ] ; sample = i*BS + j*128 + p
        neg_raw += rowsums.T.reshape(B)
        pos_half[i * BS : (i + 1) * BS] = possums.T.reshape(BS)

    e2 = np.exp(INV_TEMP)  # exp(1/T * 1.0) diagonal term
    neg = (neg_raw - D * e2) / (B - 1)
    pos = 2.0 * pos_half
    logits = pos / (pos + neg)
    m = logits.max()
    lse = np.log(np.sum(np.exp(logits - m))) + m
    loss = lse - logits.mean()
    return np.float32(loss)


def kernel(**inputs) -> np.ndarray:
    res = _run(inputs["z"], trace=False)
    return _finish(res.results)
